# revision 10
# baseline (speedup 1.0000x reference)
"""Multi-head attention kernel for 8 Trainium2 NeuronCores.

Problem: x[4,2048,768] -> qkv proj (w_qkv[768,2304]) -> 12-head attention
(head_dim 64) -> out proj (w_proj[768,768]).

Sharding: 8 cores, each handles one (batch, head-group-of-6) pair:
core c -> batch c//2, heads (c%2)*6 .. +6. Each core computes its 6 heads'
qkv projections, attention, and the partial output projection
sum_h attnout_h @ w_proj[h-rows]. Host sums the two half-head partials per
batch. No inter-core communication needed.

On-device layout (all matmul operands float32r -> 1 cycle/row on PE):
  xT   [768, 2048]   x[b] transposed (host-prepared)
  qT/kT [384, 2048]  features on partitions (3 blocks of 128 = 6 heads)
  v    16 tiles [128, 768]: per head j 128-col slot: even j [v_j|ones],
       odd j [ones|v_j] (the ones columns produce softmax denominators
       replicated over 64 partitions in the same matmul as attn@V)
  Attention is computed fully transposed: S^T = K Q^T per 128-key block,
  P^T = exp(S^T / 8) via ACT (psum -> sbuf f32r), out^T accumulates
  lhsT=[v|ones] over key blocks. Softmax division happens on out^T via
  fast reciprocal + an SBUF->SBUF DMA partition shift + one DVE multiply.

PSUM: one pool, four 2-bank tags (sA, sB, poA, poB) shared by all phases so
no pool-boundary serialization exists; per-tile FIFO deps give pipelining.
"""

import numpy as np

import concourse.bass as bass
import concourse.mybir as mybir
import concourse.tile as tile
from concourse import bacc
from concourse.bass_utils import run_bass_kernel_spmd

F32 = mybir.dt.float32
F32R = mybir.dt.float32r
EXP = mybir.ActivationFunctionType.Exp
MULT = mybir.AluOpType.mult

B, N, DIM, HEADS, HD = 4, 2048, 768, 12, 64
NH = 6                 # heads per core
NPAIR = NH // 2        # head pairs per core
FQ = NH * HD           # 384 per-core q/k/v feature count
DC = DIM // 128        # 6 contraction chunks
TB = N // 128          # 16 token blocks
QH = 2                 # query halves
QHW = N // QH          # 1024
SCALE = HD ** -0.5

_cache = {}


def _build():
    nc = bacc.Bacc("TRN2", target_bir_lowering=False, debug=False)

    xT_d = nc.dram_tensor("xT", [DIM, N], F32R, kind="ExternalInput")
    wq_d = nc.dram_tensor("wq", [DIM, FQ], F32R, kind="ExternalInput")
    wk_d = nc.dram_tensor("wk", [DIM, FQ], F32R, kind="ExternalInput")
    wv_d = nc.dram_tensor("wv", [DIM, FQ], F32R, kind="ExternalInput")
    bq_d = nc.dram_tensor("bq", [128, NPAIR], F32, kind="ExternalInput")
    bk_d = nc.dram_tensor("bk", [128, NPAIR], F32, kind="ExternalInput")
    wp_d = nc.dram_tensor("wp", [FQ, DIM], F32R, kind="ExternalInput")
    ones_d = nc.dram_tensor("ones", [128, NPAIR * HD], F32R, kind="ExternalInput")
    out_d = nc.dram_tensor("out", [N, DIM], F32, kind="ExternalOutput")

    with tile.TileContext(nc) as tc:
        with (
            tc.tile_pool(name="persist", bufs=1) as persist,
            tc.tile_pool(name="ps", bufs=1, space="PSUM") as ps_pool,
        ):
            # cross-phase tiles: wp 9KB + qT/kT 48KB + v 48KB = 105KB/part
            wp = []
            for p in range(NPAIR):
                t = persist.tile([128, DIM], F32R, tag=f"wp{p}", name=f"wp{p}")
                nc.sync.dma_start(t[:], wp_d[p * 128:(p + 1) * 128, :])
                wp.append(t)
            qT = [persist.tile([128, N], F32R, tag=f"qT{p}", name=f"qT{p}") for p in range(NPAIR)]
            kT = [persist.tile([128, N], F32R, tag=f"kT{p}", name=f"kT{p}") for p in range(NPAIR)]
            v_sb = [persist.tile([128, 2 * FQ], F32R, tag=f"v{tb}", name=f"v{tb}") for tb in range(TB)]

            ps_tags = ["sA", "sB", "poA", "poB"]

            def ps_tile(i, width=QHW):
                return ps_pool.tile([128, width], F32, tag=ps_tags[i % 4], name=f"ps_{ps_tags[i % 4]}")

            # ================= phase 1: QKV projections ======================
            with tc.tile_pool(name="qkv_in", bufs=1) as qkv_in:
                # interleave x/weight chunk DMAs so dc-0 operands land first
                xT, wq, wk, wv = [], [], [], []
                for dc in range(DC):
                    t = qkv_in.tile([128, N], F32R, tag=f"xT{dc}", name=f"xT{dc}")
                    nc.sync.dma_start(t[:], xT_d[dc * 128:(dc + 1) * 128, :])
                    xT.append(t)
                    for (lst, src, nm) in ((wv, wv_d, "wv"), (wq, wq_d, "wq"), (wk, wk_d, "wk")):
                        t = qkv_in.tile([128, FQ], F32R, tag=f"{nm}{dc}", name=f"{nm}{dc}")
                        nc.sync.dma_start(t[:], src[dc * 128:(dc + 1) * 128, :])
                        lst.append(t)
                bq = qkv_in.tile([128, NPAIR], F32, tag="bq")
                nc.sync.dma_start(bq[:], bq_d[:])
                bk = qkv_in.tile([128, NPAIR], F32, tag="bk")
                nc.sync.dma_start(bk[:], bk_d[:])

                # V first (tags sA/sB) so attention's S pipeline frees up early
                for tb in range(TB):
                    ps = ps_tile(tb % 2, FQ)
                    for dc in range(DC):
                        nc.tensor.matmul(
                            ps[:],
                            xT[dc][:, tb * 128:(tb + 1) * 128],
                            wv[dc][:],
                            start=(dc == 0), stop=(dc == DC - 1),
                        )
                    t = v_sb[tb]
                    t4 = t[:].rearrange("p (g s c) -> p g s c", g=NPAIR, s=4, c=HD)
                    ps4 = ps[:].rearrange("p (g s c) -> p g s c", g=NPAIR, s=2, c=HD)
                    ones3 = ones_d[:].rearrange("p (g c) -> p g c", g=NPAIR, c=HD)
                    nc.vector.tensor_copy(t4[:, :, 0, :], ps4[:, :, 0, :])  # even-head v
                    nc.vector.tensor_copy(t4[:, :, 3, :], ps4[:, :, 1, :])  # odd-head v
                    nc.sync.dma_start(t4[:, :, 1, :], ones3)
                    nc.sync.dma_start(t4[:, :, 2, :], ones3)

                # q/k pair-major (tags poA/poB) so pair 0 attention unblocks first
                it = 0
                for fb in range(NPAIR):
                    for (dst, w_sb, b_sb) in ((qT, wq, bq), (kT, wk, bk)):
                        for h2 in range(2):
                            ps = ps_tile(2 + it % 2)
                            it += 1
                            for dc in range(DC):
                                for nn in range(QHW // 512):
                                    nc.tensor.matmul(
                                        ps[:, nn * 512:(nn + 1) * 512],
                                        w_sb[dc][:, fb * 128:(fb + 1) * 128],
                                        xT[dc][:, h2 * QHW + nn * 512:h2 * QHW + (nn + 1) * 512],
                                        start=(dc == 0), stop=(dc == DC - 1),
                                    )
                            nc.vector.tensor_scalar_add(
                                dst[fb][:, h2 * QHW:(h2 + 1) * QHW], ps[:], b_sb[:, fb:fb + 1])

            # ================= phase 2: attention ============================
            with (
                tc.tile_pool(name="attn_sb", bufs=1) as attn_sb,
                tc.tile_pool(name="p_pool", bufs=8) as p_pool,
                tc.tile_pool(name="rec_pool", bufs=4) as rec_pool,
                tc.tile_pool(name="ot_pool", bufs=3) as ot_pool,
            ):
                attnT = [attn_sb.tile([128, N], F32R, tag=f"attnT{p}", name=f"attnT{p}") for p in range(NPAIR)]

                def proj_half(tb_lo, tb_hi):
                    # partial output projection over token blocks [tb_lo, tb_hi)
                    for tb in range(tb_lo, tb_hi):
                        ps = ps_tile(2 + tb % 2, DIM)
                        for p in range(NPAIR):
                            for (lo, hi) in ((0, 512), (512, DIM)):
                                nc.tensor.matmul(
                                    ps[:, lo:hi],
                                    attnT[p][:, tb * 128:(tb + 1) * 128],
                                    wp[p][:, lo:hi],
                                    start=(p == 0), stop=(p == NPAIR - 1),
                                )
                        ot = ot_pool.tile([128, DIM], F32, tag="ot")
                        nc.vector.tensor_copy(ot[:], ps[:])
                        nc.sync.dma_start(out_d[tb * 128:(tb + 1) * 128, :], ot[:])

                for qh in range(QH):
                    qs = slice(qh * QHW, (qh + 1) * QHW)
                    for p in range(NPAIR):
                        poA = ps_tile(2)
                        poB = ps_tile(3)
                        for kb in range(TB):
                            ks = slice(kb * 128, (kb + 1) * 128)
                            # head A (rows 0:64): out^T rows 0:64, denom rows 64:128
                            sA = ps_tile(0)
                            sB = ps_tile(1)
                            for nn in range(QHW // 512):
                                nc.tensor.matmul(
                                    sA[:, nn * 512:(nn + 1) * 512],
                                    kT[p][0:64, ks],
                                    qT[p][0:64, qh * QHW + nn * 512: qh * QHW + (nn + 1) * 512],
                                    start=True, stop=True,
                                )
                            pA = p_pool.tile([128, QHW], F32R, tag="pt", name="pA")
                            nc.scalar.activation(pA[:], sA[:], EXP, scale=SCALE)
                            # head B (rows 64:128): denom rows 0:64, out^T rows 64:128
                            for nn in range(QHW // 512):
                                nc.tensor.matmul(
                                    sB[:, nn * 512:(nn + 1) * 512],
                                    kT[p][64:128, ks],
                                    qT[p][64:128, qh * QHW + nn * 512: qh * QHW + (nn + 1) * 512],
                                    start=True, stop=True,
                                )
                            pB = p_pool.tile([128, QHW], F32R, tag="pt", name="pB")
                            nc.scalar.activation(pB[:], sB[:], EXP, scale=SCALE)
                            for nn in range(QHW // 512):
                                nc.tensor.matmul(
                                    poA[:, nn * 512:(nn + 1) * 512],
                                    v_sb[kb][:, (2 * p) * 128:(2 * p + 1) * 128],
                                    pA[:, nn * 512:(nn + 1) * 512],
                                    start=(kb == 0), stop=(kb == TB - 1),
                                )
                            for nn in range(QHW // 512):
                                nc.tensor.matmul(
                                    poB[:, nn * 512:(nn + 1) * 512],
                                    v_sb[kb][:, (2 * p + 1) * 128:(2 * p + 2) * 128],
                                    pB[:, nn * 512:(nn + 1) * 512],
                                    start=(kb == 0), stop=(kb == TB - 1),
                                )
                        # softmax division (fast reciprocal: denom is positive,
                        # well-scaled; ~18 correct bits is ample)
                        # full-tile recip (custom DVE op breaks on partition-offset
                        # slices); the non-denominator half is overwritten by the
                        # partition-shift DMA before the multiply reads it.
                        recA = rec_pool.tile([128, QHW], F32, tag="rec", name="recA")
                        nc.vector.reciprocal_approx_fast(recA[:], poA[:])
                        nc.sync.dma_start(recA[0:64, :], recA[64:128, :])
                        nc.vector.tensor_tensor(attnT[p][0:64, qs], poA[0:64, :], recA[0:64, :], MULT)
                        recB = rec_pool.tile([128, QHW], F32, tag="rec", name="recB")
                        nc.vector.reciprocal_approx_fast(recB[:], poB[:])
                        nc.sync.dma_start(recB[64:128, :], recB[0:64, :])
                        nc.vector.tensor_tensor(attnT[p][64:128, qs], poB[64:128, :], recB[64:128, :], MULT)

                    # project the token half whose attnT columns are complete;
                    # overlaps the next qh round's S/exp stream on the PE
                    proj_half(qh * (TB // 2), (qh + 1) * (TB // 2))

    nc.compile()
    return nc


def _get_nc():
    if "nc" not in _cache:
        _cache["nc"] = _build()
    return _cache["nc"]


def make_in_maps(x, w_qkv, b_qkv, w_proj, b_proj):
    x = np.asarray(x, dtype=np.float32)
    w_qkv = np.asarray(w_qkv, dtype=np.float32)
    b_qkv = np.asarray(b_qkv, dtype=np.float32)
    w_proj = np.asarray(w_proj, dtype=np.float32)
    ones = np.ones((128, NPAIR * HD), dtype=np.float32)
    in_maps = []
    for c in range(8):
        b = c // 2
        h0 = (c % 2) * NH
        f0 = h0 * HD
        in_maps.append({
            "xT": np.ascontiguousarray(x[b].T),
            "wq": np.ascontiguousarray(w_qkv[:, f0:f0 + FQ]),
            "wk": np.ascontiguousarray(w_qkv[:, DIM + f0:DIM + f0 + FQ]),
            "wv": np.ascontiguousarray(w_qkv[:, 2 * DIM + f0:2 * DIM + f0 + FQ]),
            "bq": np.ascontiguousarray(b_qkv[f0:f0 + FQ].reshape(NPAIR, 128).T),
            "bk": np.ascontiguousarray(b_qkv[DIM + f0:DIM + f0 + FQ].reshape(NPAIR, 128).T),
            "wp": np.ascontiguousarray(w_proj[f0:f0 + FQ, :]),
            "ones": ones,
        })
    return in_maps


def combine(results, b_qkv, b_proj, w_proj):
    b_qkv = np.asarray(b_qkv, dtype=np.float32)
    b_proj = np.asarray(b_proj, dtype=np.float32)
    w_proj = np.asarray(w_proj, dtype=np.float32)
    # exact v-bias correction: attnout_h gains +bv_h, so out gains bv @ w_proj
    bias = b_proj + b_qkv[2 * DIM:] @ w_proj
    out = np.empty((B, N, DIM), dtype=np.float32)
    for b in range(B):
        out[b] = results[2 * b]["out"] + results[2 * b + 1]["out"] + bias
    return out


def kernel(x, w_qkv, b_qkv, w_proj, b_proj):
    nc = _get_nc()
    in_maps = make_in_maps(x, w_qkv, b_qkv, w_proj, b_proj)
    res = run_bass_kernel_spmd(nc, in_maps, core_ids=list(range(8)))
    return combine(res.results, b_qkv, b_proj, w_proj)
